# revision 18
# baseline (speedup 1.0000x reference)
"""Trainium2 Bass kernel: 50-step conditional diffusion sampler (CounterfactualDiffusion).

Sharding: pure data parallel — batch 8192 split as 1024 rows per NeuronCore
across 8 cores, MLP weights replicated.

Device strategy (per core):
  * Activations are kept feature-major ([feature_partition, batch_free]); the
    latent state x ([128, 1024]) stays resident in SBUF for all 50 steps.
  * Each MLP layer computes h^T = W^T @ a^T with nc.tensor.matmul
    (out = lhsT.T @ rhs), contraction tiled to K=128 chunks accumulated in
    PSUM, moving dim N=512 (two batch halves). Matmuls run as float32r
    (1 cycle/row at N>=256; plain fp32 is 4 cycles/row).
  * Layers 2/3 run gelu(psum + bias) on ScalarE directly PSUM -> SBUF; the
    step-invariant cond @ W1c is hoisted out of the loop, so layer 1 is a
    single K=128 matmul plus a VectorE add of (cond_proj + b1e[t]).
  * Matmuls are emitted k-major (contraction chunk outer) over one-bank
    [128,512] PSUM tiles, so each layer's matmuls start as soon as the
    previous layer's first gelu chunk lands (wavefront, no layer barrier).
  * The x update x' = (c2*x + sn) - c1c2*(h3@W4) runs as two
    scalar_tensor_tensor ops on VectorE.

Host-side precompute (exact, input-independent where it matters):
  * The per-step noise is jax threefry on key 42 folded with t — independent
    of all inputs — so it is generated bit-exactly with jax on CPU,
    pre-scaled by sqrt(beta_t) (zeroed at t=0), and the -c1c2_t*b4 term is
    folded in. Streamed from HBM, 512KB per step per core.
  * The t-embedding and target columns of W1 contribute a per-step constant
    vector, folded with b1 into a per-step bias table b1e[t].
"""

import os
import numpy as np

# The concourse repo is on PYTHONPATH in this container; fall back to the
# known install locations if not.
try:
    import concourse.bass as bass  # noqa: F401
except ImportError:  # pragma: no cover
    import sys

    for _p in ("/opt/trn_rl_repo", "/root/.axon_site/_ro/trn_rl_repo"):
        if os.path.isdir(_p) and _p not in sys.path:
            sys.path.insert(0, _p)
    import concourse.bass as bass  # noqa: F401

import concourse.bacc as bacc
import concourse.tile as tile
from concourse import mybir
from concourse.bass_utils import run_bass_kernel_spmd

P = 128
N_CORES = 8
BATCH = 8192
BC = BATCH // N_CORES  # 1024 batch rows per core
HALF = 512  # matmul moving-dim tile (one PSUM bank of fp32)
LATENT = 128
HIDDEN = 512
KJ = HIDDEN // P  # 4 feature chunks of the hidden dim
N_STEPS = 50

F32 = mybir.dt.float32
F32R = mybir.dt.float32r
GELU = mybir.ActivationFunctionType.Gelu
MULT = mybir.AluOpType.mult
ADD = mybir.AluOpType.add

# Set by test harness to capture a profiled run.
TRACE = False
LAST_RESULTS = None

_PROGRAM_CACHE: dict = {}


def _cpu_device():
    import jax

    return jax.devices("cpu")[0]


def _schedule(n_steps):
    """betas/alphas/alphas_cumprod exactly as the reference computes them (f32, jax)."""
    import jax
    import jax.numpy as jnp

    with jax.default_device(_cpu_device()):
        betas = jnp.linspace(1e-4, 0.02, n_steps).astype(jnp.float32)
        alphas = 1.0 - betas
        acp = jnp.cumprod(alphas)
        return (np.asarray(betas), np.asarray(alphas), np.asarray(acp))


def _step_coeffs(n_steps):
    """Per-step (s-ordered, s: 0..n-1 <-> t = n-1-s) scalar coefficients.

    c2[t]   = 1/sqrt(alpha_t)
    c1c2[t] = beta_t / sqrt(1 - acp_t) / sqrt(alpha_t)
    """
    betas, alphas, acp = _schedule(n_steps)
    bd = betas.astype(np.float64)
    ad = alphas.astype(np.float64)
    acd = acp.astype(np.float64)
    c2 = 1.0 / np.sqrt(ad)
    c1c2 = bd / np.sqrt(1.0 - acd) * c2
    c2_s, nc1c2_s = [], []
    for s in range(n_steps):
        t = n_steps - 1 - s
        c2_s.append(float(np.float32(c2[t])))
        nc1c2_s.append(float(np.float32(-c1c2[t])))
    return c2_s, nc1c2_s, betas, acp, c1c2


def _build_program(n_steps):
    """One SPMD Bass program; each core runs it on its own batch shard."""
    c2_s, nc1c2_s, _, _, _ = _step_coeffs(n_steps)

    nc = bacc.Bacc(None, target_bir_lowering=False)

    xT = nc.dram_tensor("xT", [P, BC], F32R, kind="ExternalInput")
    condT = nc.dram_tensor("condT", [P, BC], F32R, kind="ExternalInput")
    w1x = nc.dram_tensor("w1x", [P, HIDDEN], F32R, kind="ExternalInput")
    w1c = nc.dram_tensor("w1c", [P, HIDDEN], F32R, kind="ExternalInput")
    w2 = nc.dram_tensor("w2", [HIDDEN, HIDDEN], F32R, kind="ExternalInput")
    w3 = nc.dram_tensor("w3", [HIDDEN, HIDDEN], F32R, kind="ExternalInput")
    w4 = nc.dram_tensor("w4", [HIDDEN, LATENT], F32R, kind="ExternalInput")
    b1e = nc.dram_tensor("b1e", [P, KJ * n_steps], F32, kind="ExternalInput")
    b23 = nc.dram_tensor("b23", [P, 2 * KJ], F32, kind="ExternalInput")
    sn = nc.dram_tensor("sn", [n_steps, P, BC], F32, kind="ExternalInput")
    out = nc.dram_tensor("out_xT", [P, BC], F32, kind="ExternalOutput")

    with tile.TileContext(nc) as tc:
        with (
            tc.tile_pool(name="const", bufs=1) as cpool,
            tc.tile_pool(name="xpool", bufs=2) as xpool,
            tc.tile_pool(name="noise", bufs=3) as npool,
            tc.tile_pool(name="hpool", bufs=8) as hpool,
            tc.tile_pool(name="prepool", bufs=4) as prepool,
            tc.tile_pool(name="upool", bufs=2) as upool,
            tc.tile_pool(name="psum", bufs=8, space="PSUM") as ppool,
        ):
            # ---- resident constants (cond/w1c first: they gate cond_proj) ----
            w1c_sb = cpool.tile([P, HIDDEN], F32R, tag="w1c")
            nc.sync.dma_start(out=w1c_sb[:, :], in_=w1c[:, :])
            cond_sb = cpool.tile([P, BC], F32R, tag="cond")
            nc.sync.dma_start(out=cond_sb[:, :], in_=condT[:, :])
            w1x_sb = cpool.tile([P, HIDDEN], F32R, tag="w1x")
            nc.sync.dma_start(out=w1x_sb[:, :], in_=w1x[:, :])
            x_cur = xpool.tile([P, BC], F32R, tag="x")
            nc.sync.dma_start(out=x_cur[:, :], in_=xT[:, :])
            w2_sb = []
            w3_sb = []
            w4_sb = []
            for k in range(KJ):
                t2 = cpool.tile([P, HIDDEN], F32R, tag=f"w2_{k}")
                nc.sync.dma_start(out=t2[:, :], in_=w2[k * P : (k + 1) * P, :])
                w2_sb.append(t2)
                t3 = cpool.tile([P, HIDDEN], F32R, tag=f"w3_{k}")
                nc.sync.dma_start(out=t3[:, :], in_=w3[k * P : (k + 1) * P, :])
                w3_sb.append(t3)
                t4 = cpool.tile([P, LATENT], F32R, tag=f"w4_{k}")
                nc.sync.dma_start(out=t4[:, :], in_=w4[k * P : (k + 1) * P, :])
                w4_sb.append(t4)
            b1e_sb = cpool.tile([P, KJ * n_steps], F32, tag="b1e")
            nc.sync.dma_start(out=b1e_sb[:, :], in_=b1e[:, :])
            b23_sb = cpool.tile([P, 2 * KJ], F32, tag="b23")
            nc.sync.dma_start(out=b23_sb[:, :], in_=b23[:, :])

            # One-time: cond_proj = W1c.T @ condT  (step-invariant part of L1)
            cond_proj = []
            for j in range(KJ):
                cp_t = cpool.tile([P, BC], F32, tag=f"cp_{j}")
                jw = slice(j * P, (j + 1) * P)
                for hf in range(2):
                    cols = slice(hf * HALF, (hf + 1) * HALF)
                    cps = ppool.tile([P, HALF], F32, tag="ps")
                    nc.tensor.matmul(
                        cps[:, :], w1c_sb[:, jw], cond_sb[:, cols],
                        start=True, stop=True,
                    )
                    nc.vector.tensor_copy(cp_t[:, cols], cps[:, :])
                cond_proj.append(cp_t)

            for s in range(n_steps):
                sn_sb = npool.tile([P, BC], F32, tag="sn")
                nc.sync.dma_start(out=sn_sb[:, :], in_=sn[s, :, :])
                x_next = xpool.tile([P, BC], F32R, tag="x")

                # ---- layer 1: gelu(x@W1x + cond_proj + b1e[t]) ----
                h1 = []
                for hf in range(2):
                    cols = slice(hf * HALF, (hf + 1) * HALF)
                    h = hpool.tile([P, KJ * HALF], F32R, tag="h")
                    pre = prepool.tile([P, KJ * HALF], F32, tag="pre")
                    for j in range(KJ):
                        jc = slice(j * HALF, (j + 1) * HALF)
                        jw = slice(j * P, (j + 1) * P)
                        bi = KJ * s + j
                        ps = ppool.tile([P, HALF], F32, tag="ps", name="ps")
                        nc.tensor.matmul(
                            ps[:, :], w1x_sb[:, jw], x_cur[:, cols],
                            start=True, stop=True,
                        )
                        nc.vector.scalar_tensor_tensor(
                            pre[:, jc], ps[:, :], b1e_sb[:, bi : bi + 1],
                            cond_proj[j][:, cols], ADD, ADD,
                        )
                        nc.scalar.activation(h[:, jc], pre[:, jc], GELU)
                    h1.append(h)

                # Partial update (independent of the MLP): u = c2*x + sn.
                # Emitted after L1 so the DVE runs L1's bias-adds first.
                u = upool.tile([P, BC], F32, tag="u")
                nc.vector.scalar_tensor_tensor(
                    u[:, :], x_cur[:, :].bitcast(F32), c2_s[s], sn_sb[:, :], MULT, ADD
                )

                # ---- layers 2,3: gelu(h@W + b) ----
                h_prev = h1
                for li, (w_sb, boff) in enumerate(((w2_sb, 0), (w3_sb, KJ))):
                    h_new = []
                    for hf in range(2):
                        h = hpool.tile([P, KJ * HALF], F32R, tag="h")
                        ps_list = [
                            ppool.tile([P, HALF], F32, tag="ps", name="ps")
                            for _ in range(KJ)
                        ]
                        # k-major wavefront: consume h_prev chunk k as soon as
                        # its gelu lands; all 4 output chunks accumulate in step.
                        for k in range(KJ):
                            kc = slice(k * HALF, (k + 1) * HALF)
                            for j in range(KJ):
                                jw = slice(j * P, (j + 1) * P)
                                nc.tensor.matmul(
                                    ps_list[j][:, :],
                                    w_sb[k][:, jw],
                                    h_prev[hf][:, kc],
                                    start=(k == 0),
                                    stop=(k == KJ - 1),
                                )
                        for j in range(KJ):
                            jc = slice(j * HALF, (j + 1) * HALF)
                            bi = boff + j
                            nc.scalar.activation(
                                h[:, jc], ps_list[j][:, :], GELU, bias=b23_sb[:, bi : bi + 1]
                            )
                        h_new.append(h)
                    h_prev = h_new

                # ---- layer 4 + state update: x' = u - c1c2*(h3@W4) ----
                for hf in range(2):
                    pc = slice(hf * HALF, (hf + 1) * HALF)
                    ps4 = ppool.tile([P, HALF], F32, tag="ps", name="ps")
                    for k in range(KJ):
                        kc = slice(k * HALF, (k + 1) * HALF)
                        nc.tensor.matmul(
                            ps4[:, :],
                            w4_sb[k][:, :],
                            h_prev[hf][:, kc],
                            start=(k == 0),
                            stop=(k == KJ - 1),
                        )
                    nc.vector.scalar_tensor_tensor(
                        x_next[:, pc], ps4[:, :], nc1c2_s[s], u[:, pc], MULT, ADD
                    )

                x_cur = x_next

            nc.sync.dma_start(out=out[:, :], in_=x_cur[:, :].bitcast(F32))

    nc.finalize()
    return nc


def _get_program(n_steps):
    if n_steps not in _PROGRAM_CACHE:
        _PROGRAM_CACHE[n_steps] = _build_program(n_steps)
    return _PROGRAM_CACHE[n_steps]


def _noise_table(n_steps):
    """Raw N(0,1) noise per step, exactly as the reference draws it (jax CPU)."""
    import jax

    out = np.empty((n_steps, BATCH, LATENT), np.float32)
    with jax.default_device(_cpu_device()):
        key = jax.random.key(42)
        for t in range(n_steps):
            if t == 0:
                out[t] = 0.0  # reference zeroes noise at t=0
            else:
                out[t] = np.asarray(
                    jax.random.normal(jax.random.fold_in(key, t), (BATCH, LATENT))
                )
    return out


def _prepare_inputs(
    condition, x_init, W1, b1, W2, b2, W3, b3, W4, b4, target_survival, n_steps
):
    f32 = lambda a: np.ascontiguousarray(np.asarray(a, dtype=np.float32))
    condition, x_init = f32(condition), f32(x_init)
    W1, b1, W2, b2 = f32(W1), f32(b1), f32(W2), f32(b2)
    W3, b3, W4, b4 = f32(W3), f32(b3), f32(W4), f32(b4)
    target = 0.0 if np.asarray(target_survival).item() else 1.0

    _, _, betas, acp, c1c2 = _step_coeffs(n_steps)
    bd = betas.astype(np.float64)
    noise = _noise_table(n_steps)

    # s-ordered scaled noise with the -c1c2*b4 term folded in:
    #   sn[s] = sqrt(beta_t)*noise_t - c1c2_t*b4     (t = n-1-s)
    sn_full = np.empty((n_steps, BATCH, LATENT), np.float32)
    b1e = np.empty((n_steps, HIDDEN), np.float64)
    for s in range(n_steps):
        t = n_steps - 1 - s
        sn_full[s] = noise[t] * np.sqrt(bd[t]) - c1c2[t] * b4[None, :]
        t_emb = np.float64(np.float32(t) / np.float32(n_steps))
        b1e[s] = (
            b1.astype(np.float64)
            + t_emb * W1[2 * LATENT].astype(np.float64)
            + target * W1[2 * LATENT + 1].astype(np.float64)
        )
    b1e_arr = np.ascontiguousarray(
        b1e.astype(np.float32).reshape(n_steps, KJ, P).transpose(2, 0, 1).reshape(P, KJ * n_steps)
    )
    b23 = np.ascontiguousarray(
        np.concatenate([b2.reshape(KJ, P).T, b3.reshape(KJ, P).T], axis=1)
    )
    w1x = np.ascontiguousarray(W1[:LATENT])
    w1c = np.ascontiguousarray(W1[LATENT : 2 * LATENT])

    in_maps = []
    for c in range(N_CORES):
        r0, r1 = c * BC, (c + 1) * BC
        in_maps.append(
            dict(
                xT=np.ascontiguousarray(x_init[r0:r1].T),
                condT=np.ascontiguousarray(condition[r0:r1].T),
                w1x=w1x,
                w1c=w1c,
                w2=W2,
                w3=W3,
                w4=W4,
                b1e=b1e_arr,
                b23=b23,
                sn=np.ascontiguousarray(sn_full[:, r0:r1, :].transpose(0, 2, 1)),
            )
        )
    return in_maps


def _run(inputs, n_steps):
    global LAST_RESULTS
    nc = _get_program(n_steps)
    in_maps = _prepare_inputs(n_steps=n_steps, **inputs)
    res = run_bass_kernel_spmd(
        nc, in_maps, core_ids=list(range(N_CORES)), trace=TRACE
    )
    LAST_RESULTS = res
    out = np.concatenate(
        [np.asarray(res.results[c]["out_xT"]).T for c in range(N_CORES)], axis=0
    )
    return np.ascontiguousarray(out.astype(np.float32))


def kernel(
    condition,
    x_init,
    W1,
    b1,
    W2,
    b2,
    W3,
    b3,
    W4,
    b4,
    target_survival,
):
    return _run(
        dict(
            condition=condition,
            x_init=x_init,
            W1=W1,
            b1=b1,
            W2=W2,
            b2=b2,
            W3=W3,
            b3=b3,
            W4=W4,
            b4=b4,
            target_survival=target_survival,
        ),
        N_STEPS,
    )
